# revision 72
# baseline (speedup 1.0000x reference)
"""Bass/Trainium2 kernel for nn_GAT_GCN (GAT -> GCN -> cross-attention -> MLP).

Sharding: 8 cores, each owns 128 consecutive graphs (batch is sorted, so a
contiguous node slab). Edges are assigned to the core owning their dst node,
so all segment reductions are core-local.

v2 layout: the node table [x | 1 | 0 | asrc | adst] (bf16, asrc/adst = x @
(W_h a_h) folded on the host) is replicated to every core as an input, so
the GAT phase needs NO AllGather: one indirect gather per edge-tile serves
the aggregation, the src attention logits, and (via a per-window
mask-transpose matmul) the dst logits. The only collective is the AllGather
of the GAT output h (bf16), split into AG_CHUNKS contiguous row-blocks
(chunk-major global row layout) so the first chunk overlaps the tail of the
GAT compute. All heavy matmuls run in bf16 (1 PE cycle/row vs 4 for fp32);
segment sums/softmax-denominators are 0/1-mask matmuls; cross-attention
scores are fused into the GCN window loop; graph-max pooling gathers are
range-restricted so they interleave with the attended-value stores.

Algebraic structure:
  * GAT aggregates in x-space: z_h = sum_e exp(a_eh) [x|1][src_e], softmax
    denominator from the appended ones-column, normalization applied to z
    before the per-head Wcat matmul so the bias row folds in exactly.
  * GCN normalization: dinv[src] pre-scaled into the AllGathered h rows,
    dinv[dst] applied after aggregation; biases fold in via ones-columns /
    ones-row matmuls.
  * Softmax max-subtraction dropped (logits are O(1); it cancels exactly).
"""
import os
import sys
import numpy as np

sys.path.insert(0, "/opt/trn_rl_repo")

import ml_dtypes

BF = ml_dtypes.bfloat16

N, E, B = 40000, 160000, 1024
FXD, H, DOUT, DP = 78, 10, 128, 256
NCORES, GPC = 8, 128          # graphs per core
P = 128
NPAD = 5376                   # padded nodes per core (42 * 128)
NW = NPAD // P                # node windows per core
NTOT = NCORES * NPAD          # padded global node space
HF = H * FXD                  # 780
XW = FXD + 2                  # 80: [x | 1 | 0]
TW = XW + 2 * H               # 100: [x | 1 | 0 | asrc | adst]
ZW = H * XW                   # 800, z-psum width
SLOTS = 64                    # pooling slots per graph
KC, NKC = 112, 7              # gcn/fcg1 K-chunk size/count (7*112=784>=781)
DUMMY0 = NPAD                 # pooling pad slots point at dummy -inf rows
AG_CHUNKS = int(os.environ.get("KAGCH", "2"))   # h AllGather chunk count


# ----------------------------------------------------------------------------
# host preprocessing
# ----------------------------------------------------------------------------

def _host_prep(inputs):
    x = np.ascontiguousarray(np.asarray(inputs["x"], dtype=np.float32))
    edge_index = np.asarray(inputs["edge_index"]).astype(np.int64)
    batch = np.asarray(inputs["batch"]).astype(np.int64)
    pvec = np.asarray(inputs["protein_vec"], dtype=np.float32)

    src = np.concatenate([edge_index[0], np.arange(N, dtype=np.int64)])
    dst = np.concatenate([edge_index[1], np.arange(N, dtype=np.int64)])
    order = np.argsort(dst, kind="stable")
    src, dst = src[order], dst[order]

    node_core = batch // GPC
    n0 = np.searchsorted(node_core, np.arange(NCORES))
    n1 = np.searchsorted(node_core, np.arange(NCORES), side="right")
    ncount = n1 - n0
    assert ncount.max() <= NPAD, ncount.max()

    # chunk-major global row layout so each h-AllGather chunk lands in a
    # contiguous slice: row(c, l) for chunk k (windows [w0,w1)) =
    # w0*P*NCORES + c*(w1-w0)*P + (l - w0*P)
    wsplit = [round(NW * (i + 1) / AG_CHUNKS) for i in range(AG_CHUNKS)]
    wstart = [0] + wsplit[:-1]

    def row_of(c, l):
        w = l // P
        k = np.searchsorted(np.asarray(wsplit), w, side="right")
        w0, w1 = wstart[k], wsplit[k]
        return w0 * P * NCORES + c * (w1 - w0) * P + (l - w0 * P)

    row_of_v = np.vectorize(row_of, otypes=[np.int64])
    pad_id = np.zeros(N, np.int64)
    for c in range(NCORES):
        pad_id[n0[c]:n1[c]] = row_of_v(c, np.arange(ncount[c]))

    deg = np.bincount(dst, minlength=N).astype(np.float32)

    # attention projection vectors (host fold): asrc/adst = x @ (W_h a_h)
    gat_w_f = np.asarray(inputs["gat_w"], np.float32)
    a_s_f = np.asarray(inputs["gat_att_src"], np.float32)
    a_d_f = np.asarray(inputs["gat_att_dst"], np.float32)
    vs = np.stack([gat_w_f[:, h * FXD:(h + 1) * FXD] @ a_s_f[h]
                   for h in range(H)], axis=1)          # [78, H]
    vd = np.stack([gat_w_f[:, h * FXD:(h + 1) * FXD] @ a_d_f[h]
                   for h in range(H)], axis=1)          # [78, H]
    asrc_full = x @ vs                                   # [N, H]
    adst_full = x @ vd                                   # [N, H]

    # replicated padded node table [NTOT, TW] bf16: [x | 1 | 0 | asrc | adst]
    # (chunk-major row layout, same as h2s_full)
    x_rep = np.zeros((NTOT, TW), np.float32)
    for c in range(NCORES):
        rows = pad_id[n0[c]:n1[c]]
        x_rep[rows, :FXD] = x[n0[c]:n1[c]]
        x_rep[rows, FXD] = 1.0
        x_rep[rows, XW:XW + H] = asrc_full[n0[c]:n1[c]]
        x_rep[rows, XW + H:TW] = adst_full[n0[c]:n1[c]]
    x_rep = x_rep.astype(BF)

    # per (core, window) edge lists; shared tile schedule = max over cores
    e_start = np.searchsorted(dst, n0)
    e_end = np.searchsorted(dst, n1)
    win_edges = [[None] * NW for _ in range(NCORES)]
    tiles_per_win = np.zeros(NW, np.int64)
    for c in range(NCORES):
        es, ed = src[e_start[c]:e_end[c]], dst[e_start[c]:e_end[c]]
        loc = ed - n0[c]
        wbound = np.searchsorted(loc, np.arange(NW + 1) * P)
        for w in range(NW):
            a, b = int(wbound[w]), int(wbound[w + 1])
            win_edges[c][w] = (es[a:b], loc[a:b])
            tiles_per_win[w] = max(tiles_per_win[w], (b - a + P - 1) // P)
    ttot = int(tiles_per_win.sum())
    tile_start = np.zeros(NW + 1, np.int64)
    for w in range(NW):
        tile_start[w + 1] = tile_start[w] + tiles_per_win[w]

    meta = dict(ttot=ttot, tiles_per_win=[int(v) for v in tiles_per_win],
                tile_start=[int(v) for v in tile_start],
                pool_whi=[0] * (GPC * SLOTS // P))

    # weight folds (host: tiny)
    gat_b = np.asarray(inputs["gat_b"], np.float32)
    wcat = np.zeros((XW, HF), np.float32)
    for h in range(H):
        wcat[:FXD, h * FXD:(h + 1) * FXD] = gat_w_f[:, h * FXD:(h + 1) * FXD]
        wcat[FXD, h * FXD:(h + 1) * FXD] = gat_b[h * FXD:(h + 1) * FXD]

    def kchunk(wm, kc, nk, ncols, todt=BF):      # [K, ncols] -> [kc, nk*ncols]
        wp = np.zeros((kc * nk, ncols), np.float32)
        wp[:wm.shape[0]] = wm
        out = np.zeros((kc, nk * ncols), np.float32)
        for k in range(nk):
            out[:, k * ncols:(k + 1) * ncols] = wp[k * kc:(k + 1) * kc]
        return out.astype(todt)

    # fcg1 weight with bias row appended at row HF (ones col in drg)
    fcg1_wb = np.concatenate(
        [np.asarray(inputs["fcg1_w"], np.float32),
         np.asarray(inputs["fcg1_b"], np.float32)[None, :]], axis=0)  # [781,128]

    shared = {
        "wcat": wcat.astype(BF),
        "gcnw": kchunk(np.asarray(inputs["gcn_w"], np.float32), KC, NKC, HF),
        "gcnb": np.asarray(inputs["gcn_b"], np.float32)[None, :].astype(BF),
        "fcg1w": kchunk(fcg1_wb, KC, NKC, DOUT),
        "pfcw": kchunk(np.asarray(inputs["pfc_w"], np.float32), P, 2, DP,
                       np.float32),
        "pfcb": np.asarray(inputs["pfc_b"], np.float32)[None, :],
        "qw": np.asarray(inputs["q_w"], np.float32).astype(BF),
        "qb": np.asarray(inputs["q_b"], np.float32)[None, :].astype(BF),
        "kw": kchunk(np.asarray(inputs["k_w"], np.float32), P, 2, P,
                     np.float32),
        "kbr": np.asarray(inputs["k_b"], np.float32)[None, :],
        "vw": kchunk(np.asarray(inputs["v_w"], np.float32), P, 2, P,
                     np.float32),
        "vbr": np.asarray(inputs["v_b"], np.float32)[None, :],
        "fc1w": kchunk(np.asarray(inputs["fc1_w"], np.float32), P, 3, 1024),
        "fc1b": np.asarray(inputs["fc1_b"], np.float32)[None, :].astype(BF),
        "fc2w": kchunk(np.asarray(inputs["fc2_w"], np.float32), P, 8, 512),
        "fc2b": np.asarray(inputs["fc2_b"], np.float32)[None, :].astype(BF),
        "outw": kchunk(np.asarray(inputs["out_w"], np.float32), P, 4, 1),
        "outb": np.asarray(inputs["out_b"], np.float32)[None, :].astype(BF),
        "iota": np.tile(np.arange(P, dtype=np.float32), (P, 1)).astype(BF),
        "identb": np.eye(P, dtype=np.float32).astype(BF),
        "x_rep": x_rep,
    }

    in_maps = []
    for c in range(NCORES):
        nloc = int(ncount[c])
        src_idx = np.zeros((P, ttot), np.int32)
        dstcol = np.full((P, ttot), 999.0, np.float32)
        for w in range(NW):
            es, loc = win_edges[c][w]
            ne = len(es)
            for j in range(int(tiles_per_win[w])):
                t = int(tile_start[w]) + j
                a, b = j * P, min((j + 1) * P, ne)
                if a >= ne:
                    continue
                m = b - a
                src_idx[:m, t] = pad_id[es[a:b]]
                dstcol[:m, t] = (loc[a:b] - w * P).astype(np.float32)

        # node-major [P, NW*H]: adst of node w*P+p at [p, w*H:(w+1)*H]
        adst_flat = np.zeros((NPAD, H), np.float32)
        adst_flat[:nloc] = adst_full[n0[c]:n1[c]]
        adst_loc = adst_flat.reshape(NW, P, H).transpose(1, 0, 2) \
                            .reshape(P, NW * H)

        # node-major [P, NW] arrays: node w*P+p at [p, w]
        deg_flat = np.ones(NPAD, np.float32)
        deg_flat[:nloc] = deg[n0[c]:n1[c]]
        deg_local = deg_flat.reshape(NW, P).T.copy()
        bc_flat = np.full(NPAD, -1.0, np.float32)
        bc_flat[:nloc] = (batch[n0[c]:n1[c]] - c * GPC).astype(np.float32)
        batchcol = bc_flat.reshape(NW, P).T.copy()

        # pooling slots: flat slot g*SLOTS+s -> att_dense row (dummy rows
        # live at 0..P-1, node l at row P+l so gathers can start before all
        # windows are written)
        flat = np.arange(GPC * SLOTS, dtype=np.int64) % P
        bl = batch[n0[c]:n1[c]] - c * GPC
        gstart = np.searchsorted(bl, np.arange(GPC + 1))
        for g in range(GPC):
            a, b = int(gstart[g]), int(gstart[g + 1])
            m = min(b - a, SLOTS)
            flat[g * SLOTS:g * SLOTS + m] = P + np.arange(a, a + m)
        idx_pool = flat.reshape(GPC * SLOTS // P, P).T.astype(np.int32).copy()
        # highest window touched by each pooling gather column (shared
        # schedule => max over cores taken below)
        pw_hi = ((flat.reshape(GPC * SLOTS // P, P).max(1) - P) // P)
        pw_hi = np.maximum(pw_hi, 0)

        pv_slab = pvec[c * GPC:(c + 1) * GPC, 0, :]          # [128, 256]
        pvT = np.zeros((P, 2 * P), np.float32)
        for k in range(2):
            pvT[:, k * P:(k + 1) * P] = pv_slab[:, k * P:(k + 1) * P].T

        meta["pool_whi"] = [max(a, int(b))
                            for a, b in zip(meta["pool_whi"], pw_hi)]

        im = {"adst_loc": adst_loc.astype(BF), "src_idx": src_idx,
              "dstcol": dstcol, "deg_local": deg_local, "batchcol": batchcol,
              "idx_pool": idx_pool, "pvt": pvT}
        im.update(shared)
        in_maps.append({k: np.ascontiguousarray(v) for k, v in im.items()})

    return in_maps, meta


# ----------------------------------------------------------------------------
# device program
# ----------------------------------------------------------------------------

def _build_program(meta):
    from concourse import bass, bacc, mybir, tile

    f32 = mybir.dt.float32
    bf16 = mybir.dt.bfloat16
    i32 = mybir.dt.int32
    AF = mybir.ActivationFunctionType
    OP = mybir.AluOpType
    AX = mybir.AxisListType
    IOA = bass.IndirectOffsetOnAxis

    ttot = meta["ttot"]
    tpw = meta["tiles_per_win"]
    tstart = meta["tile_start"]

    nc = bacc.Bacc("TRN2", target_bir_lowering=False, debug=False,
                   enable_asserts=False, num_devices=NCORES)

    def din(name, shape, dt=bf16):
        return nc.dram_tensor(name, list(shape), dt, kind="ExternalInput").ap()

    x_rep = din("x_rep", (NTOT, TW))
    d_in = {
        "adst_loc": din("adst_loc", (P, NW * H)),
        "src_idx": din("src_idx", (P, ttot), i32),
        "dstcol": din("dstcol", (P, ttot), f32),
        "deg_local": din("deg_local", (P, NW), f32),
        "batchcol": din("batchcol", (P, NW), f32),
        "idx_pool": din("idx_pool", (P, SLOTS), i32),
        "pvt": din("pvt", (P, 2 * P), f32),
        "wcat": din("wcat", (XW, HF)),
        "gcnb": din("gcnb", (1, HF)),
        "pfcw": din("pfcw", (P, 2 * DP), f32),
        "pfcb": din("pfcb", (1, DP), f32),
        "qw": din("qw", (P, P)),
        "qb": din("qb", (1, P)),
        "kw": din("kw", (P, 2 * P), f32),
        "kbr": din("kbr", (1, P), f32),
        "vw": din("vw", (P, 2 * P), f32),
        "vbr": din("vbr", (1, P), f32),
        "iota": din("iota", (P, P)),
        "identb": din("identb", (P, P)),
    }
    gcnw_d = din("gcnw", (KC, NKC * HF))
    fcg1w_d = din("fcg1w", (KC, NKC * DOUT))
    fc1w_d = din("fc1w", (P, 3 * 1024))
    fc1b_d = din("fc1b", (1, 1024))
    fc2w_d = din("fc2w", (P, 8 * 512))
    fc2b_d = din("fc2b", (1, 512))
    outw_d = din("outw", (P, 4))
    outb_d = din("outb", (1, 1))

    y = nc.dram_tensor("y", [GPC, 1], f32, kind="ExternalOutput").ap()

    with tile.TileContext(nc) as tc:
      with tc.tile_pool(name="cst", bufs=1) as cst, \
           tc.tile_pool(name="dram", bufs=1, space="DRAM") as dram:

        def cload(pool, ap_):
            t = pool.tile(list(ap_.shape), ap_.dtype, tag=ap_.tensor.name)
            nc.sync.dma_start(out=t[:], in_=ap_)
            return t

        C = {k: cload(cst, v) for k, v in d_in.items()}
        iota, identb = C["iota"], C["identb"]

        ones_b = cst.tile([1, P], bf16)
        nc.vector.memset(ones_b[:], 1.0)
        ones_f = cst.tile([1, P], f32)
        nc.vector.memset(ones_f[:], 1.0)
        degs = cst.tile([P, NW], f32)
        dinv_all = cst.tile([P, NW], f32)
        nc.scalar.sqrt(degs[:], C["deg_local"][:])
        nc.vector.reciprocal(dinv_all[:], degs[:])

        ag_h_in = dram.tile([NPAD, HF], bf16)
        h2s_full = dram.tile([NTOT, HF], bf16)
        att_dense = dram.tile([NPAD + P, DOUT], bf16)
        adst_all = C["adst_loc"]

        rg = [list(range(NCORES))]

        def mm(out, lhsT, rhs, start, stop):
            nc.tensor.matmul(out, lhsT, rhs, start=start, stop=stop)

        # ------------------------------------------------------------------
        # Phase B: GAT -> h2s (bf16, rows pre-scaled by dinv), AllGather
        # ------------------------------------------------------------------
        # chunk boundaries for the h AllGather
        wsplit = [round(NW * (i + 1) / AG_CHUNKS) for i in range(AG_CHUNKS)]

        with tc.tile_pool(name="phb_g", bufs=6) as gp, \
             tc.tile_pool(name="phb_w", bufs=6) as wk, \
             tc.tile_pool(name="phb_z", bufs=2, space="PSUM") as psz, \
             tc.tile_pool(name="phb_t", bufs=3, space="PSUM") as pstp, \
             tc.tile_pool(name="phb_u", bufs=1, space="PSUM") as psu:
            wdone = 0
            for w in range(NW):
                nt = tpw[w]
                if nt:
                    pz = psz.tile([P, 1024], f32, tag="pz")
                    for j in range(nt):
                        t = tstart[w] + j
                        xg = gp.tile([P, TW], bf16, tag="xg")
                        nc.gpsimd.indirect_dma_start(
                            out=xg[:], out_offset=None, in_=x_rep,
                            in_offset=IOA(ap=C["src_idx"][:, t:t + 1], axis=0))
                        msk = wk.tile([P, P], bf16, tag="msk")
                        nc.vector.tensor_scalar(
                            out=msk[:], in0=iota[:],
                            scalar1=C["dstcol"][:, t:t + 1],
                            scalar2=None, op0=OP.is_equal)
                        pmt = pstp.tile([P, P], bf16, tag="t")
                        nc.tensor.transpose(pmt[:], msk[:], identb[:])
                        mskT = wk.tile([P, P], bf16, tag="mskT")
                        nc.scalar.copy(mskT[:], pmt[:])
                        pal = psu.tile([P, H], f32, tag="u")
                        mm(pal[:], mskT[:],
                           adst_all[:, w * H:(w + 1) * H], True, True)
                        al = wk.tile([P, H], f32, tag="al")
                        nc.vector.tensor_tensor(
                            out=al[:], in0=pal[:], in1=xg[:, XW:XW + H],
                            op=OP.add)
                        lr = wk.tile([P, H], bf16, tag="lr")
                        nc.vector.scalar_tensor_tensor(
                            out=lr[:], in0=al[:], scalar=0.2, in1=al[:],
                            op0=OP.mult, op1=OP.max)
                        ea = wk.tile([P, H], bf16, tag="ea")
                        nc.scalar.activation(ea[:], lr[:], AF.Exp)
                        xgs = wk.tile([P, H, XW], bf16, tag="xgs")
                        nc.vector.tensor_tensor(
                            out=xgs[:],
                            in0=xg[:, 0:XW].unsqueeze(1)
                                .broadcast_to([P, H, XW]),
                            in1=ea[:].unsqueeze(2).broadcast_to([P, H, XW]),
                            op=OP.mult)
                        xf = xgs[:].rearrange("p a b -> p (a b)")
                        mm(pz[:, 0:512], msk[:], xf[:, 0:512],
                           j == 0, j == nt - 1)
                        mm(pz[:, 512:ZW], msk[:], xf[:, 512:ZW],
                           j == 0, j == nt - 1)
                    stmp = wk.tile([P, H], f32, tag="stmp")
                    nc.vector.tensor_scalar(
                        out=stmp[:], in0=pz[:, FXD:ZW:XW], scalar1=1e-30,
                        scalar2=None, op0=OP.add)
                    sinv = wk.tile([P, H], f32, tag="sinv")
                    nc.vector.reciprocal(sinv[:], stmp[:])
                    zn = wk.tile([P, H, XW], bf16, tag="zn")
                    nc.vector.tensor_tensor(
                        out=zn[:],
                        in0=pz[:, 0:ZW].rearrange("p (a b) -> p a b", a=H),
                        in1=sinv[:].unsqueeze(2).broadcast_to([P, H, XW]),
                        op=OP.mult)
                    znf = zn[:].rearrange("p a b -> p (a b)")
                    psW = psz.tile([P, 1024], f32, tag="pz")
                    for h in range(H):
                        pzt = pstp.tile([P, P], bf16, tag="t")
                        nc.tensor.transpose(
                            pzt[0:XW, :], znf[:, h * XW:(h + 1) * XW],
                            identb[:])
                        zt = wk.tile([XW, P], bf16, tag="zt")
                        if h % 2 == 0:
                            nc.scalar.copy(zt[:], pzt[0:XW, :])
                        else:
                            nc.vector.tensor_copy(zt[:], pzt[0:XW, :])
                        off = 0 if h < 5 else 512
                        hh = h if h < 5 else h - 5
                        mm(psW[:, off + hh * FXD:off + (hh + 1) * FXD],
                           zt[:], C["wcat"][:, h * FXD:(h + 1) * FXD],
                           True, True)
                    h2sb = wk.tile([P, HF], bf16, tag="h2sb")
                    nc.scalar.activation(h2sb[:, 0:5 * FXD], psW[:, 0:5 * FXD],
                                         AF.Relu, scale=dinv_all[:, w:w + 1])
                    nc.scalar.activation(h2sb[:, 5 * FXD:HF],
                                         psW[:, 512:512 + 5 * FXD],
                                         AF.Relu, scale=dinv_all[:, w:w + 1])
                    nc.sync.dma_start(out=ag_h_in[w * P:(w + 1) * P, :],
                                      in_=h2sb[:])
                if w + 1 in wsplit:
                    r0, r1 = wdone * P, (w + 1) * P
                    nc.gpsimd.collective_compute(
                        "AllGather", OP.bypass,
                        ins=[ag_h_in[r0:r1, :]],
                        outs=[h2s_full[r0 * NCORES:r1 * NCORES, :]],
                        replica_groups=rg)
                    wdone = w + 1

        # persistent attention-phase buffers
        with tc.tile_pool(name="att", bufs=1) as att:
            dnT = att.tile([P, NPAD], bf16)
            e_all = att.tile([P, NW], f32)
            e_allb = att.tile([P, NW], bf16)
            nc.vector.memset(dnT[:], 0.0)
            nc.vector.memset(e_all[:], 0.0)

            # protein path (independent of the graph phases)
            pv_sb = att.tile([P, DP], f32)
            pvt2 = att.tile([P, 2 * P], bf16)
            pvt2f = att.tile([P, 2 * P], f32)
            k_sb = att.tile([P, P], bf16)
            v_sb = att.tile([P, P], bf16)
            identf = att.tile([P, P], f32)
            nc.vector.tensor_copy(identf[:], identb[:])
            with tc.tile_pool(name="prot_ps", bufs=2, space="PSUM") as ps:
                ppv = ps.tile([P, DP], f32, tag="ppv")
                for k in range(2):
                    mm(ppv[:], C["pvt"][:, k * P:(k + 1) * P],
                       C["pfcw"][:, k * DP:(k + 1) * DP], k == 0, False)
                mm(ppv[:], ones_f[:], C["pfcb"][:], False, True)
                nc.scalar.activation(pv_sb[:], ppv[:], AF.Relu)
                for k in range(2):
                    pt = ps.tile([P, P], f32, tag="pt")
                    nc.tensor.transpose(pt[:], pv_sb[:, k * P:(k + 1) * P],
                                        identf[:])
                    nc.vector.tensor_copy(pvt2f[:, k * P:(k + 1) * P], pt[:])
                nc.vector.tensor_copy(pvt2[:], pvt2f[:])
                for dst_t, wname, bname in ((k_sb, "kw", "kbr"),
                                            (v_sb, "vw", "vbr")):
                    pk = ps.tile([P, P], f32, tag="pk")
                    for k in range(2):
                        mm(pk[:], pvt2f[:, k * P:(k + 1) * P],
                           C[wname][:, k * P:(k + 1) * P], k == 0, False)
                    mm(pk[:], ones_f[:], C[bname][:], False, True)
                    nc.vector.tensor_copy(dst_t[:], pk[:])

            # --------------------------------------------------------------
            # Phase C: GCN + fcg1 (writes dnT)
            # --------------------------------------------------------------
            with tc.tile_pool(name="phc_c", bufs=1) as cc, \
                 tc.tile_pool(name="phc_g", bufs=6) as gp, \
                 tc.tile_pool(name="phc_w", bufs=4) as wk, \
                 tc.tile_pool(name="phc_a", bufs=2, space="PSUM") as psa, \
                 tc.tile_pool(name="phc_t", bufs=2, space="PSUM") as pstp, \
                 tc.tile_pool(name="phc_d", bufs=1, space="PSUM") as psd:
                gcnw = cload(cc, gcnw_d)
                fcg1w = cload(cc, fcg1w_d)
                for w in range(NW):
                    nt = tpw[w]
                    if nt == 0:
                        continue
                    t0 = tstart[w]
                    pagg = psa.tile([P, NKC * P], f32, tag="pagg")
                    for j in range(nt):
                        t = t0 + j
                        hg = gp.tile([P, HF], bf16, tag="hg")
                        nc.gpsimd.indirect_dma_start(
                            out=hg[:], out_offset=None, in_=h2s_full[:],
                            in_offset=IOA(ap=C["src_idx"][:, t:t + 1], axis=0))
                        mskd = wk.tile([P, P], bf16, tag="mskd")
                        nc.vector.tensor_scalar(
                            out=mskd[:], in0=iota[:],
                            scalar1=C["dstcol"][:, t:t + 1],
                            scalar2=None, op0=OP.is_equal)
                        mm(pagg[:, 0:512], mskd[:], hg[:, 0:512],
                           j == 0, j == nt - 1)
                        mm(pagg[:, 512:HF], mskd[:], hg[:, 512:HF],
                           j == 0, j == nt - 1)
                    asb = wk.tile([P, KC * NKC], bf16, tag="asb")
                    nc.scalar.activation(asb[:, 0:HF], pagg[:, 0:HF], AF.Copy,
                                         scale=dinv_all[:, w:w + 1])
                    nc.vector.memset(asb[:, HF:HF + 1], 1.0)
                    nc.vector.memset(asb[:, HF + 1:KC * NKC], 0.0)
                    aT = wk.tile([KC, NKC * P], bf16, tag="aT")
                    for k in range(NKC):
                        ptr = pstp.tile([P, P], bf16, tag="t")
                        nc.tensor.transpose(ptr[0:KC, :],
                                            asb[:, k * KC:(k + 1) * KC],
                                            identb[:])
                        if k % 2 == 0:
                            nc.scalar.copy(aT[:, k * P:(k + 1) * P],
                                           ptr[0:KC, :])
                        else:
                            nc.vector.tensor_copy(aT[:, k * P:(k + 1) * P],
                                                  ptr[0:KC, :])
                    pdr = psa.tile([P, NKC * P], f32, tag="pagg")
                    for k in range(NKC):
                        mm(pdr[:, 0:512], aT[:, k * P:(k + 1) * P],
                           gcnw[:, k * HF:k * HF + 512], k == 0, False)
                        mm(pdr[:, 512:HF], aT[:, k * P:(k + 1) * P],
                           gcnw[:, k * HF + 512:(k + 1) * HF],
                           k == 0, False)
                    mm(pdr[:, 0:512], ones_b[:], C["gcnb"][:, 0:512],
                       False, True)
                    mm(pdr[:, 512:HF], ones_b[:], C["gcnb"][:, 512:HF],
                       False, True)
                    drg = wk.tile([P, KC * NKC], bf16, tag="drg")
                    nc.scalar.activation(drg[:, 0:HF], pdr[:, 0:HF], AF.Relu)
                    nc.vector.memset(drg[:, HF:HF + 1], 1.0)
                    nc.vector.memset(drg[:, HF + 1:KC * NKC], 0.0)
                    drT = wk.tile([KC, NKC * P], bf16, tag="drT")
                    for k in range(NKC):
                        ptr = pstp.tile([P, P], bf16, tag="t")
                        nc.tensor.transpose(ptr[0:KC, :],
                                            drg[:, k * KC:(k + 1) * KC],
                                            identb[:])
                        if k % 2 == 0:
                            nc.scalar.copy(drT[:, k * P:(k + 1) * P],
                                           ptr[0:KC, :])
                        else:
                            nc.vector.tensor_copy(drT[:, k * P:(k + 1) * P],
                                                  ptr[0:KC, :])
                    pdn = psd.tile([P, P], f32, tag="pdn")
                    for k in range(NKC):
                        mm(pdn[:], fcg1w[:, k * P:(k + 1) * P],
                           drT[:, k * P:(k + 1) * P], k == 0, k == NKC - 1)
                    nc.scalar.activation(dnT[:, w * P:(w + 1) * P], pdn[:],
                                         AF.Relu)
                    # fused cross-attention scores for this window
                    g01t = wk.tile([P, P], bf16, tag="g01t")
                    nc.vector.tensor_scalar(
                        out=g01t[:], in0=iota[:],
                        scalar1=C["batchcol"][:, w:w + 1],
                        scalar2=None, op0=OP.is_equal)
                    pg = pstp.tile([P, P], bf16, tag="t")
                    nc.tensor.transpose(pg[:], g01t[:], identb[:])
                    gsb = wk.tile([P, P], bf16, tag="gsb")
                    nc.scalar.copy(gsb[:], pg[:])
                    pq = psd.tile([P, P], f32, tag="pq")
                    mm(pq[:], dnT[:, w * P:(w + 1) * P], C["qw"][:],
                       True, False)
                    mm(pq[:], ones_b[:], C["qb"][:], False, True)
                    pkb = psd.tile([P, P], f32, tag="pdn")
                    mm(pkb[:], gsb[:], k_sb[:], True, True)
                    kbs = wk.tile([P, P], f32, tag="kbs")
                    nc.scalar.copy(kbs[:], pkb[:])
                    qkb = wk.tile([P, P], f32, tag="qkb")
                    nc.vector.tensor_tensor(out=qkb[:], in0=pq[:],
                                            in1=kbs[:], op=OP.mult)
                    scb = wk.tile([P, 1], f32, tag="scb")
                    nc.vector.reduce_sum(out=scb[:], in_=qkb[:], axis=AX.X)
                    nc.scalar.activation(e_all[:, w:w + 1], scb[:], AF.Exp,
                                         scale=float(1.0 / np.sqrt(128.0)))

            # --------------------------------------------------------------
            # Phase D: cross attention + pooling + MLP
            # --------------------------------------------------------------
            with tc.tile_pool(name="phd_c", bufs=1) as cc, \
                 tc.tile_pool(name="phd_w", bufs=3) as wk:
                fc1w = cload(cc, fc1w_d)
                fc1b = cload(cc, fc1b_d)
                fc2w = cload(cc, fc2w_d)
                fc2b = cload(cc, fc2b_d)
                outw = cload(cc, outw_d)
                outb = cload(cc, outb_d)

                sinv_g = att.tile([P, 1], f32)
                v_scl = att.tile([P, P], bf16)
                pooledT = att.tile([P, P], bf16)

                with tc.tile_pool(name="phd_ps2", bufs=2, space="PSUM") as ps:
                    # graph softmax sums
                    pss = ps.tile([P, 1], f32, tag="acc")
                    nc.vector.tensor_copy(e_allb[:], e_all[:])
                    for b in range(NW):
                        g01t = wk.tile([P, P], bf16, tag="g01t")
                        nc.vector.tensor_scalar(
                            out=g01t[:], in0=iota[:],
                            scalar1=C["batchcol"][:, b:b + 1],
                            scalar2=None, op0=OP.is_equal)
                        mm(pss[:], g01t[:], e_allb[:, b:b + 1],
                           b == 0, b == NW - 1)

                    ssum = wk.tile([P, 1], f32, tag="ssum")
                    nc.vector.tensor_scalar(out=ssum[:], in0=pss[:],
                                            scalar1=1e-30, scalar2=None,
                                            op0=OP.add)
                    nc.vector.reciprocal(sinv_g[:], ssum[:])
                    nc.vector.tensor_scalar_mul(v_scl[:], v_sb[:], sinv_g[:])
                    ninf = wk.tile([P, DOUT], bf16, tag="ninf")
                    nc.vector.memset(ninf[:], -1e30)
                    nc.sync.dma_start(out=att_dense[0:P, :], in_=ninf[:])

                    # pass 2: attended -> att_dense
                    for b in range(NW):
                        g01s = wk.tile([P, P], bf16, tag="g01t")
                        nc.vector.tensor_scalar(
                            out=g01s[:], in0=iota[:],
                            scalar1=C["batchcol"][:, b:b + 1],
                            scalar2=e_all[:, b:b + 1],
                            op0=OP.is_equal, op1=OP.mult)
                        pg = ps.tile([P, P], bf16, tag="s")
                        nc.tensor.transpose(pg[:], g01s[:], identb[:])
                        g01sT = wk.tile([P, P], bf16, tag="gsb")
                        nc.scalar.copy(g01sT[:], pg[:])
                        pvb = ps.tile([P, P], f32, tag="pq")
                        mm(pvb[:], g01sT[:], v_scl[:], True, True)
                        vab = wk.tile([P, P], bf16, tag="vab")
                        nc.scalar.copy(vab[:], pvb[:])
                        pdt = ps.tile([P, P], bf16, tag="s")
                        nc.tensor.transpose(pdt[:], dnT[:, b * P:(b + 1) * P],
                                            identb[:])
                        attb = wk.tile([P, P], bf16, tag="attb")
                        nc.vector.tensor_tensor(out=attb[:], in0=pdt[:],
                                                in1=vab[:], op=OP.add)
                        nc.sync.dma_start(
                            out=att_dense[P + b * P:P + (b + 1) * P, :],
                            in_=attb[:])

                    # pooling (gathers restricted to the written row prefix
                    # so they interleave with the pass-2 stores)
                    NPT = GPC * SLOTS // P
                    for tp in range(NPT):
                        rhi = P + (meta["pool_whi"][tp] + 1) * P
                        pgt = wk.tile([P, DOUT], bf16, tag="pgt")
                        nc.gpsimd.indirect_dma_start(
                            out=pgt[:], out_offset=None,
                            in_=att_dense[0:rhi, :],
                            in_offset=IOA(ap=C["idx_pool"][:, tp:tp + 1],
                                          axis=0))
                        ppt = ps.tile([P, P], bf16, tag="s")
                        nc.tensor.transpose(ppt[:], pgt[:], identb[:])
                        g0 = tp * P // SLOTS
                        nc.vector.reduce_max(out=pooledT[:, g0:g0 + 1],
                                             in_=ppt[:, 0:SLOTS], axis=AX.X)
                        nc.vector.reduce_max(out=pooledT[:, g0 + 1:g0 + 2],
                                             in_=ppt[:, SLOTS:P], axis=AX.X)
                    pmask = wk.tile([P, P], bf16, tag="pmask")
                    nc.vector.tensor_scalar(out=pmask[:], in0=pooledT[:],
                                            scalar1=-1e29, scalar2=None,
                                            op0=OP.is_ge)
                    pooled0 = att.tile([P, P], bf16)
                    nc.vector.tensor_tensor(out=pooled0[:], in0=pooledT[:],
                                            in1=pmask[:], op=OP.mult)

                    # MLP
                    h1 = att.tile([P, 1024], bf16)
                    for hh in range(2):
                        ph1 = ps.tile([P, 512], f32, tag="ph")
                        for k in range(3):
                            lhs = pooled0[:] if k == 0 else \
                                pvt2[:, (k - 1) * P:k * P]
                            mm(ph1[:], lhs,
                               fc1w[:, k * 1024 + hh * 512:
                                    k * 1024 + (hh + 1) * 512],
                               k == 0, False)
                        mm(ph1[:], ones_b[:], fc1b[:, hh * 512:(hh + 1) * 512],
                           False, True)
                        nc.scalar.activation(h1[:, hh * 512:(hh + 1) * 512],
                                             ph1[:], AF.Relu)
                    h1T = att.tile([P, 8 * P], bf16)
                    for k in range(8):
                        pt = ps.tile([P, P], bf16, tag="s")
                        nc.tensor.transpose(pt[:], h1[:, k * P:(k + 1) * P],
                                            identb[:])
                        nc.vector.tensor_copy(h1T[:, k * P:(k + 1) * P], pt[:])
                    ph2 = ps.tile([P, 512], f32, tag="ph")
                    for k in range(8):
                        mm(ph2[:], h1T[:, k * P:(k + 1) * P],
                           fc2w[:, k * 512:(k + 1) * 512], k == 0, False)
                    mm(ph2[:], ones_b[:], fc2b[:], False, True)
                    h2 = att.tile([P, 512], bf16)
                    nc.scalar.activation(h2[:], ph2[:], AF.Relu)
                    h2T = att.tile([P, 4 * P], bf16)
                    for k in range(4):
                        pt = ps.tile([P, P], bf16, tag="s")
                        nc.tensor.transpose(pt[:], h2[:, k * P:(k + 1) * P],
                                            identb[:])
                        nc.vector.tensor_copy(h2T[:, k * P:(k + 1) * P], pt[:])
                    po = ps.tile([P, 1], f32, tag="acc")
                    for k in range(4):
                        mm(po[:], h2T[:, k * P:(k + 1) * P], outw[:, k:k + 1],
                           k == 0, False)
                    mm(po[:], ones_b[:], outb[:], False, True)
                    ysb = wk.tile([P, 1], f32, tag="ysb")
                    nc.vector.tensor_copy(ysb[:], po[:])
                    nc.sync.dma_start(out=y, in_=ysb[:])

    nc.compile()
    return nc


_CACHE = {}


def _get_program(meta):
    key = (meta["ttot"], tuple(meta["tiles_per_win"]),
           tuple(meta["pool_whi"]), AG_CHUNKS)
    if key not in _CACHE:
        _CACHE[key] = _build_program(meta)
    return _CACHE[key]


def kernel(**inputs) -> np.ndarray:
    from concourse import bass_utils
    in_maps, meta = _host_prep(inputs)
    nc = _get_program(meta)
    res = bass_utils.run_bass_kernel_spmd(nc, in_maps, list(range(NCORES)))
    out = np.zeros((B, 1), np.float32)
    for c in range(NCORES):
        out[c * GPC:(c + 1) * GPC] = res.results[c]["y"]
    return out
